# revision 6
# baseline (speedup 1.0000x reference)
"""Trainium2 Bass kernel for a causal single-head attention module.

reference computation (per batch b):
    q = x @ Wq; k = x @ Wk; v = x @ Wv          # [s, 128]
    att = softmax(mask(q @ k.T / sqrt(1024)))   # causal
    out = att @ v                               # [s, 128]

Shapes: x [4, 4096, 1024] f32, W* [1024, 128] f32.

Distribution: 8 NeuronCores, 2 per batch.  The 8 sequence blocks (512 rows
each) of a batch are split between its two cores: core 2b owns blocks
{1,3,5,7}, core 2b+1 owns {0,2,4,6}.  This interleaving balances the causal
triangle AND makes the per-core instruction graph identical (SPMD): every
core runs 4 q-tiles whose key-group counts are {2,4,6,8}; the odd core's
extra (non-causal) key group per tile is zeroed via a per-core input scalar.

Each core projects Q/Kt/V for its own 2048 rows, AllGathers Kt/V with its
pair peer (8-rank AllGather, peer slab extracted with a data-driven indirect
DMA so the graph stays core-independent), and computes attention in the
"St" orientation: St[k,q] = Kt_tile.T @ Qt so that P^T = exp(St) is directly
the stationary operand of the AV matmul (no on-chip transposes of P).
Row sums come from a ones-vector matmul.  Normalisation and the final
[dv, q] -> [q, dv] transpose happen on host during unshard.
"""

import os
import numpy as np

import concourse.bass as bass
import concourse.bacc as bacc
import concourse.mybir as mybir
import concourse.tile as tile
from concourse.bass_utils import run_bass_kernel_spmd
from concourse.masks import make_identity

F32 = mybir.dt.float32
I32 = mybir.dt.int32

BATCH = 4
SEQ = 4096
EMB = 1024
DK = 128
P = 128
NCORES = 8
SCALE = 1.0 / float(np.sqrt(EMB))

# Block structure: 8 global blocks per batch, 4 per core.
NBLK = 8
HEAVY_BLOCKS = [1, 3, 5, 7]  # core 2b   (exact causal fit)
LIGHT_BLOCKS = [0, 2, 4, 6]  # core 2b+1 (one padded key-group per tile)

# order in which (tile, own/peer) work is emitted: own-first runway so the
# AllGather overlaps with own-key attention; peer groups interleaved to
# bound PSUM liveness at 3 Ot accumulators.
TILE_PHASES = [(0, 0), (1, 0), (2, 0), (0, 1), (3, 0), (1, 1), (2, 1), (3, 1)]


def build_nc(seq: int = SEQ):
    """Build the (core-independent) Bass graph for sequence length seq."""
    blk = seq // NBLK          # rows per block (512 for seq=4096)
    sub = blk // P             # 128-key subtiles per key group
    kcols = 4 * blk            # own rows per core (= seq/2)
    emb_c = EMB // P           # contraction chunks (8)

    nc = bacc.Bacc("TRN2", target_bir_lowering=False, debug=False,
                   num_devices=NCORES)

    # ---- kernel I/O ----
    xt = nc.dram_tensor("xt", [EMB, kcols], F32, kind="ExternalInput")
    wq = nc.dram_tensor("wq", [EMB, DK], F32, kind="ExternalInput")
    wk = nc.dram_tensor("wk", [EMB, DK], F32, kind="ExternalInput")
    wv = nc.dram_tensor("wv", [EMB, DK], F32, kind="ExternalInput")
    pad = nc.dram_tensor("pad", [P, 1], F32, kind="ExternalInput")
    idx_k = nc.dram_tensor("idx_k", [P, 1], I32, kind="ExternalInput")
    idx_v = nc.dram_tensor("idx_v", [P, 1], I32, kind="ExternalInput")
    out_o = nc.dram_tensor("out_o", [P, 4 * blk], F32, kind="ExternalOutput")
    out_s = nc.dram_tensor("out_s", [4, blk], F32, kind="ExternalOutput")

    # collective bounce buffers (internal DRAM)
    cc_in = nc.dram_tensor("cc_in", [2 * P, kcols], F32)
    cc_out = nc.dram_tensor("cc_out", [NCORES * 2 * P, kcols], F32,
                            addr_space="Shared")

    with tile.TileContext(nc) as tc:
        with tc.tile_pool(name="persist", bufs=1) as persist:
            # persistent SBUF tensors
            xt_sb = persist.tile([P, emb_c, kcols], F32)
            wq_sb = persist.tile([P, emb_c, DK], F32)
            wk_sb = persist.tile([P, emb_c, DK], F32)
            wv_sb = persist.tile([P, emb_c, DK], F32)
            qt_sb = persist.tile([P, 4 * blk], F32)
            kt_sb = persist.tile([P, 8 * blk], F32)
            v_sb = persist.tile([P, 8 * sub, P], F32)
            dmask = persist.tile([P, sub, blk], F32)
            ones_sb = persist.tile([P, 1], F32)
            pad_sb = persist.tile([P, 1], F32)
            idxk_sb = persist.tile([P, 1], I32)
            idxv_sb = persist.tile([P, 1], I32)
            ident = persist.tile([P, P], F32)
            sums_sb = persist.tile([1, 4 * blk], F32)

            # ---- constants / small inputs ----
            make_identity(nc, ident[:])
            nc.gpsimd.memset(ones_sb[:], 1.0)
            nc.sync.dma_start(pad_sb[:], pad.ap())
            nc.sync.dma_start(idxk_sb[:], idx_k.ap())
            nc.sync.dma_start(idxv_sb[:], idx_v.ap())
            # causal mask for the aligned-diagonal key group:
            # dmask[k, j, q] = 1.0 if q >= j*128 + k else 0.0
            nc.gpsimd.memset(dmask[:], 1.0)
            for j in range(sub):
                nc.gpsimd.affine_select(
                    out=dmask[:, j, :],
                    in_=dmask[:, j, :],
                    compare_op=mybir.AluOpType.is_ge,
                    fill=0.0,
                    base=-(j * P),
                    pattern=[[1, blk]],
                    channel_multiplier=-1,
                )

            # ---- weight + activation loads ----
            for w_dram, w_sb in ((wq, wq_sb), (wk, wk_sb), (wv, wv_sb)):
                nc.sync.dma_start(
                    w_sb[:], w_dram.ap().rearrange("(c p) d -> p c d", p=P))
            for c in range(emb_c):
                nc.sync.dma_start(xt_sb[:, c, :], xt.ap()[c * P:(c + 1) * P, :])

            nch = kcols // 512  # 512-wide column chunks of the projections

            with (
                tc.tile_pool(name="proj_psum", bufs=2, space="PSUM") as pp,
                tc.tile_pool(name="tp_psum", bufs=2, space="PSUM") as tpp,
                tc.tile_pool(name="vt_tmp", bufs=2) as vtp,
            ):
                # K^T projection: kt_sb[:, n*512...] = Wk^T @ x^T  (own half)
                for n in range(nch):
                    ps = pp.tile([P, 512], F32, tag="proj")
                    for c in range(emb_c):
                        nc.tensor.matmul(ps[:], wk_sb[:, c, :],
                                         xt_sb[:, c, n * 512:(n + 1) * 512],
                                         start=(c == 0), stop=(c == emb_c - 1))
                    nc.vector.tensor_copy(kt_sb[:, n * 512:(n + 1) * 512], ps[:])
                # V^T projection then PE-transpose into natural V tiles
                for n in range(nch):
                    ps = pp.tile([P, 512], F32, tag="proj")
                    for c in range(emb_c):
                        nc.tensor.matmul(ps[:], wv_sb[:, c, :],
                                         xt_sb[:, c, n * 512:(n + 1) * 512],
                                         start=(c == 0), stop=(c == emb_c - 1))
                    vt = vtp.tile([P, 512], F32, tag="vt")
                    nc.vector.tensor_copy(vt[:], ps[:])
                    for j in range(4):
                        tp = tpp.tile([P, P], F32, tag="tp")
                        nc.tensor.transpose(tp[:], vt[:, j * P:(j + 1) * P],
                                            ident[:])
                        nc.vector.tensor_copy(v_sb[:, n * 4 + j, :], tp[:])

                # ---- pair exchange of Kt/V via 8-rank AllGather ----
                nc.sync.dma_start(cc_in.ap()[0:P, :], kt_sb[:, 0:kcols])
                nc.sync.dma_start(cc_in.ap()[P:2 * P, :],
                                  v_sb[:, 0:4 * sub, :].rearrange('p t d -> p (t d)'))
                nc.gpsimd.collective_compute(
                    "AllGather",
                    mybir.AluOpType.bypass,
                    ins=[cc_in.ap()],
                    outs=[cc_out.ap()],
                    replica_groups=[list(range(NCORES))],
                )
                # peer slab extraction (per-core row indices -> same graph)
                nc.gpsimd.indirect_dma_start(
                    out=kt_sb[:, kcols:2 * kcols],
                    out_offset=None,
                    in_=cc_out.ap(),
                    in_offset=bass.IndirectOffsetOnAxis(ap=idxk_sb[:, :1], axis=0),
                )
                nc.gpsimd.indirect_dma_start(
                    out=v_sb[:, 4 * sub:8 * sub, :].rearrange('p t d -> p (t d)'),
                    out_offset=None,
                    in_=cc_out.ap(),
                    in_offset=bass.IndirectOffsetOnAxis(ap=idxv_sb[:, :1], axis=0),
                )

                # Q^T projection (after the collective is queued)
                for n in range(nch):
                    ps = pp.tile([P, 512], F32, tag="proj")
                    for c in range(emb_c):
                        nc.tensor.matmul(ps[:], wq_sb[:, c, :],
                                         xt_sb[:, c, n * 512:(n + 1) * 512],
                                         start=(c == 0), stop=(c == emb_c - 1))
                    nc.vector.tensor_copy(qt_sb[:, n * 512:(n + 1) * 512], ps[:])

            # ---- attention ----
            with (
                tc.tile_pool(name="st_psum", bufs=1, space="PSUM") as stp,
                tc.tile_pool(name="ot_psum", bufs=3, space="PSUM") as otp,
                tc.tile_pool(name="sum_psum", bufs=1, space="PSUM") as smp,
                tc.tile_pool(name="pt_pool", bufs=2) as ptp,
                tc.tile_pool(name="ot_sb_pool", bufs=2) as osp,
            ):
                ot_tiles = {}
                mm_done = {}
                total_mm = {i: 2 * (i + 1) * sub for i in range(4)}
                grp_done = {i: 0 for i in range(4)}
                total_grp = {i: 2 * (i + 1) for i in range(4)}

                for (i, phase) in TILE_PHASES:
                    if phase == 0:
                        slots = list(range(0, i + 1))
                        ot_tiles[i] = otp.tile([P, blk], F32, tag="ot",
                                               name=f"ot_{i}")
                        mm_done[i] = 0
                    else:
                        slots = list(range(4, 5 + i))
                    qs = qt_sb[:, i * blk:(i + 1) * blk]
                    for s in slots:
                        st = stp.tile([P, sub * blk], F32, tag="st")
                        for j in range(sub):
                            nc.tensor.matmul(
                                st[:, j * blk:(j + 1) * blk],
                                kt_sb[:, s * blk + j * P: s * blk + (j + 1) * P],
                                qs,
                                start=True, stop=True)
                        pt = ptp.tile([P, sub * blk], F32, tag="pt")
                        nc.scalar.activation(pt[:], st[:],
                                             mybir.ActivationFunctionType.Exp,
                                             bias=0.0, scale=SCALE)
                        if s == i:  # aligned diagonal group
                            nc.vector.tensor_tensor(
                                pt[:], pt[:],
                                dmask[:].rearrange("p s b -> p (s b)"),
                                mybir.AluOpType.mult)
                        if s == 4 + i:  # pad group (zeroed on light cores)
                            nc.vector.tensor_scalar_mul(pt[:], pt[:],
                                                        pad_sb[:, 0:1])
                        # Ot accumulation: Ot[dv, q] += V_tile.T @ Pt
                        for j in range(sub):
                            nc.tensor.matmul(
                                ot_tiles[i][:],
                                v_sb[:, s * sub + j, :],
                                pt[:, j * blk:(j + 1) * blk],
                                start=(mm_done[i] == 0),
                                stop=(mm_done[i] == total_mm[i] - 1))
                            mm_done[i] += 1
                        # row sums via ones-vector matmul
                        sm = smp.tile([1, blk], F32, tag="sm")
                        for j in range(sub):
                            nc.tensor.matmul(
                                sm[:], ones_sb[:, 0:1],
                                pt[:, j * blk:(j + 1) * blk],
                                start=(j == 0), stop=(j == sub - 1))
                        ss = sums_sb[0:1, i * blk:(i + 1) * blk]
                        if grp_done[i] == 0:
                            nc.vector.tensor_copy(ss, sm[:])
                        else:
                            nc.vector.tensor_tensor(ss, ss, sm[:],
                                                    mybir.AluOpType.add)
                        grp_done[i] += 1
                        if grp_done[i] == total_grp[i]:
                            ot_out = osp.tile([P, blk], F32, tag="ot_sb")
                            nc.vector.tensor_copy(ot_out[:], ot_tiles[i][:])
                            nc.sync.dma_start(
                                out_o.ap()[:, i * blk:(i + 1) * blk],
                                ot_out[:])
                            nc.sync.dma_start(
                                out_s.ap()[i:i + 1, :],
                                sums_sb[0:1, i * blk:(i + 1) * blk])

    nc.compile()
    return nc


_NC_CACHE = {}


def _get_nc(seq: int):
    if seq not in _NC_CACHE:
        _NC_CACHE[seq] = build_nc(seq)
    return _NC_CACHE[seq]


def make_in_maps(x, Wq, Wk, Wv, seq=None):
    """Host-side sharding: build the 8 per-core input maps."""
    x = np.asarray(x, dtype=np.float32)
    Wq = np.asarray(Wq, dtype=np.float32)
    Wk = np.asarray(Wk, dtype=np.float32)
    Wv = np.asarray(Wv, dtype=np.float32)
    seq = seq or x.shape[1]
    blk = seq // NBLK
    in_maps = []
    for core in range(NCORES):
        b, h = core // 2, core % 2
        blocks = HEAVY_BLOCKS if h == 0 else LIGHT_BLOCKS
        rows = np.concatenate(
            [np.arange(g * blk, (g + 1) * blk) for g in blocks])
        xt = np.ascontiguousarray(x[b].T[:, rows])
        peer = core ^ 1
        idxk = (peer * 2 * P + np.arange(P, dtype=np.int32)).reshape(P, 1)
        idxv = (peer * 2 * P + P + np.arange(P, dtype=np.int32)).reshape(P, 1)
        padv = np.full((P, 1), 1.0 if h == 0 else 0.0, dtype=np.float32)
        in_maps.append({
            "xt": xt, "wq": Wq, "wk": Wk, "wv": Wv,
            "pad": padv, "idx_k": idxk.astype(np.int32),
            "idx_v": idxv.astype(np.int32),
        })
    return in_maps


def unshard(results, seq=None, batch=BATCH):
    seq = seq or SEQ
    blk = seq // NBLK
    out = np.empty((batch, seq, DK), dtype=np.float32)
    for core in range(NCORES):
        b, h = core // 2, core % 2
        blocks = HEAVY_BLOCKS if h == 0 else LIGHT_BLOCKS
        oo = np.asarray(results[core]["out_o"])  # [128, 4*blk]
        ss = np.asarray(results[core]["out_s"])  # [4, blk]
        for i, g in enumerate(blocks):
            o_cols = oo[:, i * blk:(i + 1) * blk]        # [dv, blk]
            out[b, g * blk:(g + 1) * blk, :] = (o_cols / ss[i][None, :]).T
    return out


LAST_EXEC_NS = None
LAST_RESULTS = None


def kernel(x, Wq, Wk, Wv):
    global LAST_EXEC_NS, LAST_RESULTS
    x = np.asarray(x, dtype=np.float32)
    seq = x.shape[1]
    nc = _get_nc(seq)
    in_maps = make_in_maps(x, Wq, Wk, Wv, seq)
    trace = bool(os.environ.get("BASS_KERNEL_TRACE"))
    res = run_bass_kernel_spmd(nc, in_maps, core_ids=list(range(NCORES)),
                               trace=trace)
    LAST_EXEC_NS = res.exec_time_ns
    LAST_RESULTS = res
    return unshard(res.results, seq, x.shape[0])


if __name__ == "__main__":
    rng = np.random.default_rng(0)
    x = rng.standard_normal((BATCH, SEQ, EMB), dtype=np.float32)
    Wq = rng.standard_normal((EMB, DK), dtype=np.float32) / 32
    Wk = rng.standard_normal((EMB, DK), dtype=np.float32) / 32
    Wv = rng.standard_normal((EMB, DK), dtype=np.float32) / 32
    out = kernel(x, Wq, Wk, Wv)
    print("out", out.shape, out.dtype, "exec_ns", LAST_EXEC_NS)


# revision 11
# speedup vs baseline: 2.1423x; 2.1423x over previous
"""Trainium2 Bass kernel for a causal single-head attention module.

reference computation (per batch b):
    q = x @ Wq; k = x @ Wk; v = x @ Wv          # [s, 128]
    att = softmax(mask(q @ k.T / sqrt(1024)))   # causal
    out = att @ v                               # [s, 128]

Shapes: x [4, 4096, 1024] f32, W* [1024, 128] f32.

Distribution: 8 NeuronCores, 2 per batch.  The 8 sequence blocks (512 rows
each) of a batch are split between its two cores: core 2b owns blocks
{1,3,5,7}, core 2b+1 owns {0,2,4,6}.  This interleaving balances the causal
triangle AND makes the per-core instruction graph identical (SPMD): every
core runs 4 q-tiles whose key-group counts are {2,4,6,8}; the odd core's
extra (non-causal) key group per tile is zeroed via a per-core input scalar.

Each core projects Q/Kt/V for its own 2048 rows, AllGathers Kt/V with its
pair peer (8-rank AllGather, peer slab extracted with a data-driven indirect
DMA so the graph stays core-independent), and computes attention in the
"St" orientation: St[k,q] = Kt_tile.T @ Qt so that P^T = exp(St) is directly
the stationary operand of the AV matmul (no on-chip transposes of P).
Row sums come from a ones-vector matmul.  Normalisation and the final
[dv, q] -> [q, dv] transpose happen on host during unshard.
"""

import os
import ml_dtypes
import numpy as np

import concourse.bass as bass
import concourse.bacc as bacc
import concourse.mybir as mybir
import concourse.tile as tile
from concourse.bass_utils import run_bass_kernel_spmd
from concourse.masks import make_identity

F32 = mybir.dt.float32
BF16 = mybir.dt.bfloat16
I32 = mybir.dt.int32

BATCH = 4
SEQ = 4096
EMB = 1024
DK = 128
P = 128
NCORES = 8
SCALE = 1.0 / float(np.sqrt(EMB))

# Block structure: 8 global blocks per batch, 4 per core.
NBLK = 8
HEAVY_BLOCKS = [1, 3, 5, 7]  # core 2b   (exact causal fit)
LIGHT_BLOCKS = [0, 2, 4, 6]  # core 2b+1 (one padded key-group per tile)

# order in which (tile, own/peer) work is emitted: own-first runway so the
# AllGather overlaps with own-key attention; peer groups interleaved to
# bound PSUM liveness at 3 Ot accumulators.
TILE_PHASES = [(0, 0), (1, 0), (2, 0), (0, 1), (3, 0), (1, 1), (2, 1), (3, 1)]


def build_nc(seq: int = SEQ):
    """Build the (core-independent) Bass graph for sequence length seq."""
    blk = seq // NBLK          # rows per block (512 for seq=4096)
    sub = blk // P             # 128-key subtiles per key group
    kcols = 4 * blk            # own rows per core (= seq/2)
    emb_c = EMB // P           # contraction chunks (8)

    nc = bacc.Bacc("TRN2", target_bir_lowering=False, debug=False,
                   num_devices=NCORES)

    # ---- kernel I/O ----
    xt = nc.dram_tensor("xt", [EMB, kcols], BF16, kind="ExternalInput")
    wq = nc.dram_tensor("wq", [EMB, DK], BF16, kind="ExternalInput")
    wk = nc.dram_tensor("wk", [EMB, DK], BF16, kind="ExternalInput")
    wv = nc.dram_tensor("wv", [EMB, DK], BF16, kind="ExternalInput")
    pad = nc.dram_tensor("pad", [P, 1], F32, kind="ExternalInput")
    idx_k = nc.dram_tensor("idx_k", [P, 1], I32, kind="ExternalInput")
    idx_v = nc.dram_tensor("idx_v", [P, 1], I32, kind="ExternalInput")
    out_o = nc.dram_tensor("out_o", [P, 4 * blk], F32, kind="ExternalOutput")
    out_s = nc.dram_tensor("out_s", [4, blk], F32, kind="ExternalOutput")

    # collective bounce buffers (internal DRAM)
    cc_in = nc.dram_tensor("cc_in", [2 * P, kcols], BF16)
    cc_out = nc.dram_tensor("cc_out", [NCORES * 2 * P, kcols], BF16,
                            addr_space="Shared")

    with tile.TileContext(nc) as tc:
        with tc.tile_pool(name="persist", bufs=1) as persist:
            # persistent SBUF tensors
            xt_sb = persist.tile([P, emb_c, kcols], BF16)
            wq_sb = persist.tile([P, emb_c, DK], BF16)
            wk_sb = persist.tile([P, emb_c, DK], BF16)
            wv_sb = persist.tile([P, emb_c, DK], BF16)
            qt_sb = persist.tile([P, 4 * blk], BF16)
            kt_sb = persist.tile([P, 8 * blk], BF16)
            v_sb = persist.tile([P, 8 * sub, P], BF16)
            dmask = persist.tile([P, sub, blk], BF16)
            ones_sb = persist.tile([P, 1], BF16)
            pad_sb = persist.tile([P, 1], F32)
            idxk_sb = persist.tile([P, 1], I32)
            idxv_sb = persist.tile([P, 1], I32)
            ident = persist.tile([P, P], BF16)
            sums_sb = persist.tile([1, 4 * blk], F32)

            # ---- constants / small inputs ----
            make_identity(nc, ident[:])
            nc.gpsimd.memset(ones_sb[:], 1.0)
            nc.sync.dma_start(pad_sb[:], pad.ap())
            nc.sync.dma_start(idxk_sb[:], idx_k.ap())
            nc.sync.dma_start(idxv_sb[:], idx_v.ap())
            # causal mask for the aligned-diagonal key group:
            # dmask[k, j, q] = 1.0 if q >= j*128 + k else 0.0
            nc.gpsimd.memset(dmask[:], 1.0)
            for j in range(sub):
                nc.gpsimd.affine_select(
                    out=dmask[:, j, :],
                    in_=dmask[:, j, :],
                    compare_op=mybir.AluOpType.is_ge,
                    fill=0.0,
                    base=-(j * P),
                    pattern=[[1, blk]],
                    channel_multiplier=-1,
                )

            # ---- weight + activation loads ----
            for w_dram, w_sb in ((wq, wq_sb), (wk, wk_sb), (wv, wv_sb)):
                nc.sync.dma_start(
                    w_sb[:], w_dram.ap().rearrange("(c p) d -> p c d", p=P))
            for c in range(emb_c):
                nc.sync.dma_start(xt_sb[:, c, :], xt.ap()[c * P:(c + 1) * P, :])

            nch = kcols // 512  # 512-wide column chunks of the projections

            with (
                tc.tile_pool(name="proj_psum", bufs=2, space="PSUM") as pp,
                tc.tile_pool(name="tp_psum", bufs=2, space="PSUM") as tpp,
                tc.tile_pool(name="vt_tmp", bufs=2) as vtp,
            ):
                # K^T projection: kt_sb[:, n*512...] = Wk^T @ x^T  (own half)
                for n in range(nch):
                    ps = pp.tile([P, 512], F32, tag="proj")
                    for c in range(emb_c):
                        nc.tensor.matmul(ps[:], wk_sb[:, c, :],
                                         xt_sb[:, c, n * 512:(n + 1) * 512],
                                         start=(c == 0), stop=(c == emb_c - 1))
                    nc.vector.tensor_copy(kt_sb[:, n * 512:(n + 1) * 512], ps[:])
                # V^T projection then PE-transpose into natural V tiles
                for n in range(nch):
                    ps = pp.tile([P, 512], F32, tag="proj")
                    for c in range(emb_c):
                        nc.tensor.matmul(ps[:], wv_sb[:, c, :],
                                         xt_sb[:, c, n * 512:(n + 1) * 512],
                                         start=(c == 0), stop=(c == emb_c - 1))
                    vt = vtp.tile([P, 512], BF16, tag="vt")
                    nc.vector.tensor_copy(vt[:], ps[:])
                    for j in range(4):
                        tp = tpp.tile([P, P], BF16, tag="tp")
                        nc.tensor.transpose(tp[:], vt[:, j * P:(j + 1) * P],
                                            ident[:])
                        nc.vector.tensor_copy(v_sb[:, n * 4 + j, :], tp[:])

                # ---- pair exchange of Kt/V via 8-rank AllGather ----
                nc.sync.dma_start(cc_in.ap()[0:P, :], kt_sb[:, 0:kcols])
                nc.sync.dma_start(cc_in.ap()[P:2 * P, :],
                                  v_sb[:, 0:4 * sub, :].rearrange('p t d -> p (t d)'))
                nc.gpsimd.collective_compute(
                    "AllGather",
                    mybir.AluOpType.bypass,
                    ins=[cc_in.ap()],
                    outs=[cc_out.ap()],
                    replica_groups=[list(range(NCORES))],
                )
                # peer slab extraction (per-core row indices -> same graph)
                nc.gpsimd.indirect_dma_start(
                    out=kt_sb[:, kcols:2 * kcols],
                    out_offset=None,
                    in_=cc_out.ap(),
                    in_offset=bass.IndirectOffsetOnAxis(ap=idxk_sb[:, :1], axis=0),
                )
                nc.gpsimd.indirect_dma_start(
                    out=v_sb[:, 4 * sub:8 * sub, :].rearrange('p t d -> p (t d)'),
                    out_offset=None,
                    in_=cc_out.ap(),
                    in_offset=bass.IndirectOffsetOnAxis(ap=idxv_sb[:, :1], axis=0),
                )

                # Q^T projection (after the collective is queued)
                for n in range(nch):
                    ps = pp.tile([P, 512], F32, tag="proj")
                    for c in range(emb_c):
                        nc.tensor.matmul(ps[:], wq_sb[:, c, :],
                                         xt_sb[:, c, n * 512:(n + 1) * 512],
                                         start=(c == 0), stop=(c == emb_c - 1))
                    nc.vector.tensor_copy(qt_sb[:, n * 512:(n + 1) * 512], ps[:])

            # ---- attention ----
            with (
                tc.tile_pool(name="st_psum", bufs=1, space="PSUM") as stp,
                tc.tile_pool(name="ot_psum", bufs=3, space="PSUM") as otp,
                tc.tile_pool(name="sum_psum", bufs=1, space="PSUM") as smp,
                tc.tile_pool(name="pt_pool", bufs=2) as ptp,
                tc.tile_pool(name="ot_sb_pool", bufs=2) as osp,
            ):
                ot_tiles = {}
                mm_done = {}
                total_mm = {i: 2 * (i + 1) * sub for i in range(4)}
                grp_done = {i: 0 for i in range(4)}
                total_grp = {i: 2 * (i + 1) for i in range(4)}

                for (i, phase) in TILE_PHASES:
                    if phase == 0:
                        slots = list(range(0, i + 1))
                        ot_tiles[i] = otp.tile([P, blk], F32, tag="ot",
                                               name=f"ot_{i}")
                        mm_done[i] = 0
                    else:
                        slots = list(range(4, 5 + i))
                    qs = qt_sb[:, i * blk:(i + 1) * blk]
                    halves = 2 if sub >= 2 else 1
                    hs = sub // halves
                    for s in slots:
                        sm = smp.tile([1, blk], F32, tag="sm")
                        for h in range(halves):
                            st = stp.tile([P, hs * blk], F32, tag="st")
                            for j in range(hs):
                                jj = h * hs + j
                                nc.tensor.matmul(
                                    st[:, j * blk:(j + 1) * blk],
                                    kt_sb[:, s * blk + jj * P:
                                          s * blk + (jj + 1) * P],
                                    qs,
                                    start=True, stop=True)
                            pt = ptp.tile([P, hs * blk], BF16, tag="pt")
                            nc.scalar.activation(
                                pt[:], st[:],
                                mybir.ActivationFunctionType.Exp,
                                bias=0.0, scale=SCALE)
                            if s == i:  # aligned diagonal group
                                nc.vector.tensor_tensor(
                                    pt[:], pt[:],
                                    dmask[:, h * hs:(h + 1) * hs, :]
                                    .rearrange("p s b -> p (s b)"),
                                    mybir.AluOpType.mult)
                            if s == 4 + i:  # pad group (zeroed on light)
                                nc.vector.tensor_scalar_mul(pt[:], pt[:],
                                                            pad_sb[:, 0:1])
                            # Ot accumulation: Ot[dv, q] += V_tile.T @ Pt
                            for j in range(hs):
                                jj = h * hs + j
                                nc.tensor.matmul(
                                    ot_tiles[i][:],
                                    v_sb[:, s * sub + jj, :],
                                    pt[:, j * blk:(j + 1) * blk],
                                    start=(mm_done[i] == 0),
                                    stop=(mm_done[i] == total_mm[i] - 1))
                                mm_done[i] += 1
                            # row sums via ones-vector matmul
                            for j in range(hs):
                                jj = h * hs + j
                                nc.tensor.matmul(
                                    sm[:], ones_sb[:, 0:1],
                                    pt[:, j * blk:(j + 1) * blk],
                                    start=(jj == 0), stop=(jj == sub - 1))
                        ss = sums_sb[0:1, i * blk:(i + 1) * blk]
                        if grp_done[i] == 0:
                            nc.vector.tensor_copy(ss, sm[:])
                        else:
                            nc.vector.tensor_tensor(ss, ss, sm[:],
                                                    mybir.AluOpType.add)
                        grp_done[i] += 1
                        if grp_done[i] == total_grp[i]:
                            ot_out = osp.tile([P, blk], F32, tag="ot_sb")
                            nc.vector.tensor_copy(ot_out[:], ot_tiles[i][:])
                            nc.sync.dma_start(
                                out_o.ap()[:, i * blk:(i + 1) * blk],
                                ot_out[:])
                            nc.sync.dma_start(
                                out_s.ap()[i:i + 1, :],
                                sums_sb[0:1, i * blk:(i + 1) * blk])

    nc.compile()
    return nc


_NC_CACHE = {}


def _get_nc(seq: int):
    if seq not in _NC_CACHE:
        _NC_CACHE[seq] = build_nc(seq)
    return _NC_CACHE[seq]


def make_in_maps(x, Wq, Wk, Wv, seq=None):
    """Host-side sharding: build the 8 per-core input maps."""
    x = np.asarray(x, dtype=np.float32)
    Wq = np.asarray(Wq, dtype=np.float32)
    Wk = np.asarray(Wk, dtype=np.float32)
    Wv = np.asarray(Wv, dtype=np.float32)
    seq = seq or x.shape[1]
    blk = seq // NBLK
    in_maps = []
    for core in range(NCORES):
        b, h = core // 2, core % 2
        blocks = HEAVY_BLOCKS if h == 0 else LIGHT_BLOCKS
        rows = np.concatenate(
            [np.arange(g * blk, (g + 1) * blk) for g in blocks])
        xt = np.ascontiguousarray(x[b].T[:, rows]).astype(ml_dtypes.bfloat16)
        peer = core ^ 1
        idxk = (peer * 2 * P + np.arange(P, dtype=np.int32)).reshape(P, 1)
        idxv = (peer * 2 * P + P + np.arange(P, dtype=np.int32)).reshape(P, 1)
        padv = np.full((P, 1), 1.0 if h == 0 else 0.0, dtype=np.float32)
        in_maps.append({
            "xt": xt,
            "wq": Wq.astype(ml_dtypes.bfloat16),
            "wk": Wk.astype(ml_dtypes.bfloat16),
            "wv": Wv.astype(ml_dtypes.bfloat16),
            "pad": padv, "idx_k": idxk.astype(np.int32),
            "idx_v": idxv.astype(np.int32),
        })
    return in_maps


def unshard(results, seq=None, batch=BATCH):
    seq = seq or SEQ
    blk = seq // NBLK
    out = np.empty((batch, seq, DK), dtype=np.float32)
    for core in range(NCORES):
        b, h = core // 2, core % 2
        blocks = HEAVY_BLOCKS if h == 0 else LIGHT_BLOCKS
        oo = np.asarray(results[core]["out_o"])  # [128, 4*blk]
        ss = np.asarray(results[core]["out_s"])  # [4, blk]
        for i, g in enumerate(blocks):
            o_cols = oo[:, i * blk:(i + 1) * blk]        # [dv, blk]
            out[b, g * blk:(g + 1) * blk, :] = (o_cols / ss[i][None, :]).T
    return out


LAST_EXEC_NS = None
LAST_RESULTS = None


def kernel(x, Wq, Wk, Wv):
    global LAST_EXEC_NS, LAST_RESULTS
    x = np.asarray(x, dtype=np.float32)
    seq = x.shape[1]
    nc = _get_nc(seq)
    in_maps = make_in_maps(x, Wq, Wk, Wv, seq)
    trace = bool(os.environ.get("BASS_KERNEL_TRACE"))
    res = run_bass_kernel_spmd(nc, in_maps, core_ids=list(range(NCORES)),
                               trace=trace)
    LAST_EXEC_NS = res.exec_time_ns
    LAST_RESULTS = res
    return unshard(res.results, seq, x.shape[0])


if __name__ == "__main__":
    rng = np.random.default_rng(0)
    x = rng.standard_normal((BATCH, SEQ, EMB), dtype=np.float32)
    Wq = rng.standard_normal((EMB, DK), dtype=np.float32) / 32
    Wk = rng.standard_normal((EMB, DK), dtype=np.float32) / 32
    Wv = rng.standard_normal((EMB, DK), dtype=np.float32) / 32
    out = kernel(x, Wq, Wk, Wv)
    print("out", out.shape, out.dtype, "exec_ns", LAST_EXEC_NS)


# revision 14
# speedup vs baseline: 3.9472x; 1.8425x over previous
"""Trainium2 Bass kernel for a causal single-head attention module.

reference computation (per batch b):
    q = x @ Wq; k = x @ Wk; v = x @ Wv          # [s, 128]
    att = softmax(mask(q @ k.T / sqrt(1024)))   # causal
    out = att @ v                               # [s, 128]

Shapes: x [4, 4096, 1024] f32, W* [1024, 128] f32.

Distribution: 8 NeuronCores, 2 per batch.  The 8 sequence blocks (512 rows
each) of a batch are split between its two cores: core 2b owns blocks
{1,3,5,7}, core 2b+1 owns {0,2,4,6}.  This interleaving balances the causal
triangle AND makes the per-core instruction graph identical (SPMD): every
core runs 4 q-tiles whose key-group counts are {2,4,6,8}; the odd core's
extra (non-causal) key group per tile is zeroed via a per-core input scalar.

Each core projects Q/Kt/V for its own 2048 rows (bf16 on the PE, fp32 PSUM),
exchanges Kt/V with its pair peer via a 2-rank AllGather (peer slab read
back with a data-driven indirect DMA so the graph stays core-independent),
and computes attention in the "St" orientation: St[k,q] = Kt_tile.T @ Qt so
that P^T = exp(St) is directly the stationary operand of the AV matmul.
Row sums use DVE partial adds + one ones-vector matmul per key group.
All own-key groups are emitted before any peer-key group (the PE stream is
a FIFO - anything behind a stalled instruction waits), with separate
own/peer Ot accumulators per q-tile to stay within the 8 PSUM banks.
Normalisation and the final [dv, q] -> [q, dv] transpose happen on host
during unshard.
"""

import os
import ml_dtypes
import numpy as np

import concourse.bass as bass
import concourse.bacc as bacc
import concourse.mybir as mybir
import concourse.tile as tile
from concourse.bass_utils import run_bass_kernel_spmd
from concourse.masks import make_identity

F32 = mybir.dt.float32
BF16 = mybir.dt.bfloat16
I32 = mybir.dt.int32

BATCH = 4
SEQ = 4096
EMB = 1024
DK = 128
P = 128
NCORES = 8
SCALE = 1.0 / float(np.sqrt(EMB))

NBLK = 8
HEAVY_BLOCKS = [1, 3, 5, 7]  # core 2b   (exact causal fit)
LIGHT_BLOCKS = [0, 2, 4, 6]  # core 2b+1 (one padded key-group per tile)


def build_nc(seq: int = SEQ):
    """Build the (core-independent) Bass graph for sequence length seq."""
    blk = seq // NBLK          # rows per block (512 for seq=4096)
    sub = blk // P             # 128-key subtiles per key group
    kcols = 4 * blk            # own rows per core (= seq/2)
    emb_c = EMB // P           # contraction chunks (8)
    nch = kcols // 512 if kcols >= 512 else 1
    chw = min(512, kcols)      # projection column-chunk width

    nc = bacc.Bacc("TRN2", target_bir_lowering=False, debug=False,
                   num_devices=NCORES)

    # ---- kernel I/O ----
    xt = nc.dram_tensor("xt", [EMB, kcols], BF16, kind="ExternalInput")
    wq = nc.dram_tensor("wq", [EMB, DK], BF16, kind="ExternalInput")
    wk = nc.dram_tensor("wk", [EMB, DK], BF16, kind="ExternalInput")
    wv = nc.dram_tensor("wv", [EMB, DK], BF16, kind="ExternalInput")
    pad = nc.dram_tensor("pad", [P, 1], F32, kind="ExternalInput")
    idx_k = nc.dram_tensor("idx_k", [P, 1], I32, kind="ExternalInput")
    idx_v = nc.dram_tensor("idx_v", [P, 1], I32, kind="ExternalInput")
    out_o = nc.dram_tensor("out_o", [P, 4 * blk], F32, kind="ExternalOutput")
    out_s = nc.dram_tensor("out_s", [4, blk], F32, kind="ExternalOutput")

    # collective bounce buffers (internal DRAM); 2-rank pair AllGather
    cc_in = nc.dram_tensor("cc_in", [2 * P, kcols], BF16)
    cc_out = nc.dram_tensor("cc_out", [2 * 2 * P, kcols], BF16)

    with tile.TileContext(nc) as tc:
        with tc.tile_pool(name="persist", bufs=1) as persist:
            xt_sb = persist.tile([P, emb_c, kcols], BF16)
            wq_sb = persist.tile([P, emb_c, DK], BF16)
            wk_sb = persist.tile([P, emb_c, DK], BF16)
            wv_sb = persist.tile([P, emb_c, DK], BF16)
            qt_sb = persist.tile([P, 4 * blk], BF16)
            kt_sb = persist.tile([P, 8 * blk], BF16)
            v_sb = persist.tile([P, 8 * sub, P], BF16)
            dmask = persist.tile([P, sub, blk], BF16)
            ones_sb = persist.tile([P, 1], BF16)
            pad_sb = persist.tile([P, 1], F32)
            idxk_sb = persist.tile([P, 1], I32)
            idxv_sb = persist.tile([P, 1], I32)
            ident = persist.tile([P, P], BF16)
            sums_sb = persist.tile([1, 4 * blk], F32)
            ot_own_sb = persist.tile([P, 4, blk], F32)
            vt_all = persist.tile([P, kcols], BF16)

            # ---- constants / small inputs ----
            make_identity(nc, ident[:])
            nc.gpsimd.memset(ones_sb[:], 1.0)
            nc.sync.dma_start(pad_sb[:], pad.ap())
            nc.sync.dma_start(idxk_sb[:], idx_k.ap())
            nc.sync.dma_start(idxv_sb[:], idx_v.ap())
            # causal mask for the aligned-diagonal key group:
            # dmask[k, j, q] = 1.0 if q >= j*128 + k else 0.0
            nc.gpsimd.memset(dmask[:], 1.0)
            for j in range(sub):
                nc.gpsimd.affine_select(
                    out=dmask[:, j, :],
                    in_=dmask[:, j, :],
                    compare_op=mybir.AluOpType.is_ge,
                    fill=0.0,
                    base=-(j * P),
                    pattern=[[1, blk]],
                    channel_multiplier=-1,
                )

            for w_dram, w_sb in ((wq, wq_sb), (wk, wk_sb), (wv, wv_sb)):
                nc.sync.dma_start(
                    w_sb[:], w_dram.ap().rearrange("(c p) d -> p c d", p=P))
            for c in range(emb_c):
                nc.sync.dma_start(xt_sb[:, c, :], xt.ap()[c * P:(c + 1) * P, :])

            # ---- K^T and V^T projections, chunk-outer so the PE trails the
            # xt DMA chunk arrivals; all 8 PSUM banks hold the accumulators.
            with tc.tile_pool(name="kv_psum", bufs=1, space="PSUM") as kvp:
                k_ps = [kvp.tile([P, chw], F32, name=f"kps_{n}")
                        for n in range(nch)]
                v_ps = [kvp.tile([P, chw], F32, name=f"vps_{n}")
                        for n in range(nch)]
                for c in range(emb_c):
                    for n in range(nch):
                        nc.tensor.matmul(k_ps[n][:], wk_sb[:, c, :],
                                         xt_sb[:, c, n * chw:(n + 1) * chw],
                                         start=(c == 0),
                                         stop=(c == emb_c - 1))
                    for n in range(nch):
                        nc.tensor.matmul(v_ps[n][:], wv_sb[:, c, :],
                                         xt_sb[:, c, n * chw:(n + 1) * chw],
                                         start=(c == 0),
                                         stop=(c == emb_c - 1))
                for n in range(nch):
                    nc.vector.tensor_copy(kt_sb[:, n * chw:(n + 1) * chw],
                                          k_ps[n][:])
                for n in range(nch):
                    nc.vector.tensor_copy(vt_all[:, n * chw:(n + 1) * chw],
                                          v_ps[n][:])

            # V^T bf16 sbuf -> PE-transpose to natural V tiles
            with tc.tile_pool(name="tp_psum", bufs=2, space="PSUM") as tpp:
                for t in range(kcols // P):
                    tp = tpp.tile([P, P], BF16, tag="tp")
                    nc.tensor.transpose(tp[:], vt_all[:, t * P:(t + 1) * P],
                                        ident[:])
                    nc.vector.tensor_copy(v_sb[:, t, :], tp[:])

            # ---- pair exchange of Kt/V via 2-rank AllGather ----
            nc.sync.dma_start(cc_in.ap()[0:P, :], kt_sb[:, 0:kcols])
            nc.sync.dma_start(cc_in.ap()[P:2 * P, :],
                              v_sb[:, 0:4 * sub, :]
                              .rearrange("p t d -> p (t d)"))
            nc.gpsimd.collective_compute(
                "AllGather",
                mybir.AluOpType.bypass,
                ins=[cc_in.ap()],
                outs=[cc_out.ap()],
                replica_groups=[[0, 1], [2, 3], [4, 5], [6, 7]],
            )
            # peer slab extraction (per-core row indices -> same graph)
            nc.gpsimd.indirect_dma_start(
                out=kt_sb[:, kcols:2 * kcols],
                out_offset=None,
                in_=cc_out.ap(),
                in_offset=bass.IndirectOffsetOnAxis(ap=idxk_sb[:, :1], axis=0),
            )
            nc.gpsimd.indirect_dma_start(
                out=v_sb[:, 4 * sub:8 * sub, :].rearrange("p t d -> p (t d)"),
                out_offset=None,
                in_=cc_out.ap(),
                in_offset=bass.IndirectOffsetOnAxis(ap=idxv_sb[:, :1], axis=0),
            )

            # ---- Q^T projection (after the collective is queued) ----
            with tc.tile_pool(name="q_psum", bufs=2, space="PSUM") as qp:
                for n in range(nch):
                    ps = qp.tile([P, chw], F32, tag="qproj")
                    for c in range(emb_c):
                        nc.tensor.matmul(ps[:], wq_sb[:, c, :],
                                         xt_sb[:, c, n * chw:(n + 1) * chw],
                                         start=(c == 0),
                                         stop=(c == emb_c - 1))
                    nc.vector.tensor_copy(qt_sb[:, n * chw:(n + 1) * chw],
                                          ps[:])

            # ---- attention: all own-key groups first, then peer groups ----
            halves = 2 if sub >= 2 else 1
            hs = sub // halves
            with (
                tc.tile_pool(name="st_psum", bufs=2, space="PSUM") as stp,
                tc.tile_pool(name="ot_psum", bufs=2, space="PSUM") as otp,
                tc.tile_pool(name="sum_psum", bufs=2, space="PSUM") as smp,
                tc.tile_pool(name="pt_pool", bufs=3) as ptp,
                tc.tile_pool(name="acc_pool", bufs=2) as accp,
                tc.tile_pool(name="ot_sb_pool", bufs=2) as osp,
            ):
                for phase in (0, 1):
                    for i in range(4):
                        slots = (list(range(0, i + 1)) if phase == 0
                                 else list(range(4, 5 + i)))
                        ot = otp.tile([P, blk], F32, tag="ot",
                                      name=f"ot_{i}_{phase}")
                        n_mm = (i + 1) * sub
                        mm = 0
                        qs = qt_sb[:, i * blk:(i + 1) * blk]
                        for s in slots:
                            pts = []
                            for h in range(halves):
                                st = stp.tile([P, hs * blk], F32, tag="st")
                                for j in range(hs):
                                    jj = h * hs + j
                                    nc.tensor.matmul(
                                        st[:, j * blk:(j + 1) * blk],
                                        kt_sb[:, s * blk + jj * P:
                                              s * blk + (jj + 1) * P],
                                        qs,
                                        start=True, stop=True)
                                pt = ptp.tile([P, hs * blk], BF16, tag="pt")
                                nc.scalar.activation(
                                    pt[:], st[:],
                                    mybir.ActivationFunctionType.Exp,
                                    bias=0.0, scale=SCALE)
                                if s == i:  # aligned diagonal group
                                    nc.vector.tensor_tensor(
                                        pt[:], pt[:],
                                        dmask[:, h * hs:(h + 1) * hs, :]
                                        .rearrange("p s b -> p (s b)"),
                                        mybir.AluOpType.mult)
                                if s == 4 + i:  # pad group (zeroed on light)
                                    nc.vector.tensor_scalar_mul(
                                        pt[:], pt[:], pad_sb[:, 0:1])
                                pts.append(pt)
                                # Ot accumulation: Ot[dv, q] += V_tile.T @ Pt
                                for j in range(hs):
                                    jj = h * hs + j
                                    nc.tensor.matmul(
                                        ot[:],
                                        v_sb[:, s * sub + jj, :],
                                        pt[:, j * blk:(j + 1) * blk],
                                        start=(mm == 0),
                                        stop=(mm == n_mm - 1))
                                    mm += 1
                            # row sums: DVE partial adds, then one matmul
                            if sub == 1:
                                acc = pts[0]
                            else:
                                acc = accp.tile([P, blk], BF16, tag="acc")
                                h0, h1 = pts
                                nc.vector.tensor_tensor(
                                    acc[:], h0[:, 0:blk], h0[:, blk:2 * blk],
                                    mybir.AluOpType.add)
                                tmp = accp.tile([P, blk], BF16, tag="acc2")
                                nc.vector.tensor_tensor(
                                    tmp[:], h1[:, 0:blk], h1[:, blk:2 * blk],
                                    mybir.AluOpType.add)
                                nc.vector.tensor_tensor(
                                    acc[:], acc[:], tmp[:],
                                    mybir.AluOpType.add)
                            sm = smp.tile([1, blk], F32, tag="sm")
                            nc.tensor.matmul(sm[:], ones_sb[:, 0:1], acc[:],
                                             start=True, stop=True)
                            ss = sums_sb[0:1, i * blk:(i + 1) * blk]
                            if phase == 0 and s == 0:
                                nc.vector.tensor_copy(ss, sm[:])
                            else:
                                nc.vector.tensor_tensor(
                                    ss, ss, sm[:], mybir.AluOpType.add)
                        if phase == 0:
                            nc.vector.tensor_copy(ot_own_sb[:, i, :], ot[:])
                        else:
                            ot_out = osp.tile([P, blk], F32, tag="ot_sb")
                            nc.vector.tensor_tensor(
                                ot_out[:], ot[:], ot_own_sb[:, i, :],
                                mybir.AluOpType.add)
                            nc.sync.dma_start(
                                out_o.ap()[:, i * blk:(i + 1) * blk],
                                ot_out[:])
                            nc.sync.dma_start(
                                out_s.ap()[i:i + 1, :],
                                sums_sb[0:1, i * blk:(i + 1) * blk])

    nc.compile()
    return nc


_NC_CACHE = {}


def _get_nc(seq: int):
    if seq not in _NC_CACHE:
        _NC_CACHE[seq] = build_nc(seq)
    return _NC_CACHE[seq]


def make_in_maps(x, Wq, Wk, Wv, seq=None):
    """Host-side sharding: build the 8 per-core input maps."""
    x = np.asarray(x, dtype=np.float32)
    Wq = np.asarray(Wq, dtype=np.float32)
    Wk = np.asarray(Wk, dtype=np.float32)
    Wv = np.asarray(Wv, dtype=np.float32)
    seq = seq or x.shape[1]
    blk = seq // NBLK
    in_maps = []
    for core in range(NCORES):
        b, h = core // 2, core % 2
        blocks = HEAVY_BLOCKS if h == 0 else LIGHT_BLOCKS
        rows = np.concatenate(
            [np.arange(g * blk, (g + 1) * blk) for g in blocks])
        xt = np.ascontiguousarray(x[b].T[:, rows]).astype(ml_dtypes.bfloat16)
        # 2-rank AllGather output: slab 0 = even rank, slab 1 = odd rank
        peer_h = 1 - h
        idxk = (peer_h * 2 * P + np.arange(P, dtype=np.int32)).reshape(P, 1)
        idxv = (peer_h * 2 * P + P + np.arange(P, dtype=np.int32)).reshape(P, 1)
        padv = np.full((P, 1), 1.0 if h == 0 else 0.0, dtype=np.float32)
        in_maps.append({
            "xt": xt,
            "wq": Wq.astype(ml_dtypes.bfloat16),
            "wk": Wk.astype(ml_dtypes.bfloat16),
            "wv": Wv.astype(ml_dtypes.bfloat16),
            "pad": padv,
            "idx_k": idxk.astype(np.int32),
            "idx_v": idxv.astype(np.int32),
        })
    return in_maps


def unshard(results, seq=None, batch=BATCH):
    seq = seq or SEQ
    blk = seq // NBLK
    out = np.empty((batch, seq, DK), dtype=np.float32)
    for core in range(NCORES):
        b, h = core // 2, core % 2
        blocks = HEAVY_BLOCKS if h == 0 else LIGHT_BLOCKS
        oo = np.asarray(results[core]["out_o"])  # [128, 4*blk]
        ss = np.asarray(results[core]["out_s"])  # [4, blk]
        for i, g in enumerate(blocks):
            o_cols = oo[:, i * blk:(i + 1) * blk]        # [dv, blk]
            out[b, g * blk:(g + 1) * blk, :] = (o_cols / ss[i][None, :]).T
    return out


LAST_EXEC_NS = None
LAST_RESULTS = None


def kernel(x, Wq, Wk, Wv):
    global LAST_EXEC_NS, LAST_RESULTS
    x = np.asarray(x, dtype=np.float32)
    seq = x.shape[1]
    nc = _get_nc(seq)
    in_maps = make_in_maps(x, Wq, Wk, Wv, seq)
    trace = bool(os.environ.get("BASS_KERNEL_TRACE"))
    res = run_bass_kernel_spmd(nc, in_maps, core_ids=list(range(NCORES)),
                               trace=trace)
    LAST_EXEC_NS = res.exec_time_ns
    LAST_RESULTS = res
    return unshard(res.results, seq, x.shape[0])


if __name__ == "__main__":
    rng = np.random.default_rng(0)
    x = rng.standard_normal((BATCH, SEQ, EMB), dtype=np.float32)
    Wq = rng.standard_normal((EMB, DK), dtype=np.float32) / 32
    Wk = rng.standard_normal((EMB, DK), dtype=np.float32) / 32
    Wv = rng.standard_normal((EMB, DK), dtype=np.float32) / 32
    out = kernel(x, Wq, Wk, Wv)
    print("out", out.shape, out.dtype, "exec_ns", LAST_EXEC_NS)
